# revision 1
# baseline (speedup 1.0000x reference)
"""TRN2 Bass kernel for the LSQ-quantized 2-layer MLP.

reference computation:
    wq1 = lsq_quant(w1, alpha1); wq2 = lsq_quant(w2, alpha2)   (tiny 256x256)
    h = relu(x @ wq1.T + b1)
    y = sigmoid(h @ wq2.T + b2)                                 x: [262144, 256] f32

Strategy: data-parallel over 8 NeuronCores (32768 tokens/core). Weight
quantization happens on the host (it is a 256x256 elementwise op, replicated
to every core). On-core pipeline per 512-token macro-tile:
    DMA x -> PE transpose (fp32, exact) -> DVE copy to f32r SBUF
    -> fc1 matmuls (f32r, w1 chunks stationary) -> hT in PSUM
    -> ACT relu+bias -> f32r SBUF
    -> fc2 matmuls (f32r, hT chunks stationary) -> y in PSUM (natural layout)
    -> ACT sigmoid -> SBUF -> DMA out
f32r matmuls run at 1 cycle/row (vs 4 for f32) with ~1e-4 relative error.
"""

import numpy as np

import concourse.bass as bass
import concourse.mybir as mybir
import concourse.tile as tile
from concourse import bacc
from concourse.bass import ts
from concourse.bass_utils import run_bass_kernel_spmd
from concourse.masks import make_identity

N_CORES = 8
N_TOK = 262144
C = 256
TOK_PER_CORE = N_TOK // N_CORES  # 32768
T_MACRO = 512
N_MACROS = TOK_PER_CORE // T_MACRO  # 64
P = 128

F32 = mybir.dt.float32
F32R = mybir.dt.float32r

_program_cache = {}


def _build_program(use_b2: bool):
    nc = bacc.Bacc("TRN2", target_bir_lowering=False, debug=False, num_devices=N_CORES)

    x_d = nc.declare_dram_parameter("x", [TOK_PER_CORE, C], F32, isOutput=False)
    w1t_d = nc.declare_dram_parameter("w1t", [P, 2, C], F32, isOutput=False)
    w2t_d = nc.declare_dram_parameter("w2t", [P, 2, C], F32, isOutput=False)
    b1s_d = nc.declare_dram_parameter("b1s", [P, 2], F32, isOutput=False)
    if use_b2:
        b2bc_d = nc.declare_dram_parameter("b2bc", [P, 512], F32, isOutput=False)
    y_d = nc.declare_dram_parameter("y", [TOK_PER_CORE, C], F32, isOutput=True)

    # token index = m*512 + g*128 + p
    x_v = x_d.rearrange("(m g p) c -> m p g c", g=4, p=P)
    y_v = y_d.rearrange("(m g p) c -> m p g c", g=4, p=P)

    with tile.TileContext(nc) as tc:
        with (
            tc.tile_pool(name="const", bufs=1) as const_pool,
            tc.tile_pool(name="sb_x", bufs=3) as sb_x,
            tc.tile_pool(name="sb_xt", bufs=3) as sb_xt,
            tc.tile_pool(name="sb_ht", bufs=3) as sb_ht,
            tc.tile_pool(name="sb_y", bufs=3) as sb_y,
            tc.tile_pool(name="ps_x", bufs=2, space="PSUM") as ps_x,
            tc.tile_pool(name="ps_h", bufs=2, space="PSUM") as ps_h,
            tc.tile_pool(name="ps_y", bufs=2, space="PSUM") as ps_y,
        ):
            ident = const_pool.tile([P, P], F32)
            make_identity(nc, ident[:])

            # weights: DMA f32 then one-time DVE cast to f32r
            w1_f32 = const_pool.tile([P, 2, C], F32)
            w2_f32 = const_pool.tile([P, 2, C], F32)
            nc.sync.dma_start(w1_f32[:], w1t_d[:])
            nc.sync.dma_start(w2_f32[:], w2t_d[:])
            w1t = const_pool.tile([P, 2, C], F32R)
            w2t = const_pool.tile([P, 2, C], F32R)
            nc.vector.tensor_copy(w1t[:], w1_f32[:])
            nc.vector.tensor_copy(w2t[:], w2_f32[:])

            b1s = const_pool.tile([P, 2], F32)
            nc.sync.dma_start(b1s[:], b1s_d[:])
            if use_b2:
                b2bc = const_pool.tile([P, 512], F32)
                nc.sync.dma_start(b2bc[:], b2bc_d[:])

            for m in range(N_MACROS):
                x_sb = sb_x.tile([P, 4, C], F32, tag="x")
                nc.sync.dma_start(x_sb[:], x_v[m])

                # transpose x -> xT (channels on partitions), f32 exact
                xt = sb_xt.tile([P, 2, T_MACRO], F32R, tag="xt")
                for c in range(2):
                    pxt = ps_x.tile([P, T_MACRO], F32, tag="pxt")
                    for g in range(4):
                        nc.tensor.transpose(
                            pxt[:, ts(g, P)], x_sb[:, g, ts(c, P)], ident[:]
                        )
                    nc.vector.tensor_copy(xt[:, c, :], pxt[:])

                # fc1: hT[j_chunk] = sum_c w1t[:,c,jchunk].T @ xT[:,c,:]
                ht = sb_ht.tile([P, 2, T_MACRO], F32R, tag="ht")
                for j in range(2):
                    pht = ps_h.tile([P, T_MACRO], F32, tag="pht")
                    for c in range(2):
                        nc.tensor.matmul(
                            pht[:],
                            w1t[:, c, ts(j, P)],
                            xt[:, c, :],
                            start=(c == 0),
                            stop=(c == 1),
                        )
                    nc.scalar.activation(
                        ht[:, j, :],
                        pht[:],
                        mybir.ActivationFunctionType.Relu,
                        bias=b1s[:, j : j + 1],
                    )

                # fc2: y[tok_chunk] = sum_c ht[:,c,tokchunk].T @ w2t[:,c,:]
                y_sb = sb_y.tile([P, 4, C], F32, tag="y")
                for half in range(2):
                    py = ps_y.tile([P, 512], F32, tag="py")
                    for tg in range(2):
                        t = half * 2 + tg
                        for c in range(2):
                            nc.tensor.matmul(
                                py[:, ts(tg, C)],
                                ht[:, c, ts(t, P)],
                                w2t[:, c, :],
                                start=(c == 0),
                                stop=(c == 1),
                            )
                    if use_b2:
                        nc.vector.tensor_add(py[:], py[:], b2bc[:])
                    nc.scalar.activation(
                        y_sb[:, half * 2 : half * 2 + 2, :],
                        py[:].rearrange("p (a b) -> p a b", a=2),
                        mybir.ActivationFunctionType.Sigmoid,
                    )
                nc.scalar.dma_start(y_v[m], y_sb[:])

    nc.compile()
    return nc


def _quantize_lsq(w: np.ndarray, alpha: np.ndarray) -> np.ndarray:
    """Replicates reference lsq_quant_weight forward numerics in np float32."""
    one = np.float32(1.0)
    g = one / np.sqrt(np.float32(w.size * 7))
    alpha = np.float32(alpha)
    a = np.float32(alpha * g) + np.float32(alpha * np.float32(one - g))
    t = np.clip((w / a).astype(np.float32), np.float32(-8.0), np.float32(7.0))
    r = (np.round(t) - t).astype(np.float32)
    q = (t + r).astype(np.float32)
    return (q * a).astype(np.float32)


def kernel(x, w1, b1, alpha1, w2, b2, alpha2):
    x = np.ascontiguousarray(np.asarray(x, dtype=np.float32))
    w1 = np.asarray(w1, dtype=np.float32)
    w2 = np.asarray(w2, dtype=np.float32)
    b1 = np.asarray(b1, dtype=np.float32)
    b2 = np.asarray(b2, dtype=np.float32)

    wq1 = _quantize_lsq(w1, np.asarray(alpha1, dtype=np.float32))
    wq2 = _quantize_lsq(w2, np.asarray(alpha2, dtype=np.float32))

    # lhsT layout for fc1: w1t[ci, co, j] = wq1[j, co*128+ci]
    w1t = np.ascontiguousarray(wq1.T.reshape(2, P, C).transpose(1, 0, 2))
    # moving operand for fc2: w2t[ci, co, j] = wq2[j, co*128+ci]
    w2t = np.ascontiguousarray(wq2.T.reshape(2, P, C).transpose(1, 0, 2))
    b1s = np.ascontiguousarray(b1.reshape(2, P).T)

    use_b2 = bool(np.any(b2))
    key = use_b2
    if key not in _program_cache:
        _program_cache[key] = _build_program(use_b2)
    nc = _program_cache[key]

    shards = np.split(x, N_CORES, axis=0)
    in_maps = []
    for s in shards:
        m = {
            "x": np.ascontiguousarray(s),
            "w1t": w1t,
            "w2t": w2t,
            "b1s": b1s,
        }
        if use_b2:
            m["b2bc"] = np.ascontiguousarray(
                np.broadcast_to(np.concatenate([b2, b2]), (P, 512))
            )
        in_maps.append(m)

    res = run_bass_kernel_spmd(nc, in_maps, list(range(N_CORES)))
    out = np.concatenate([res.results[i]["y"] for i in range(N_CORES)], axis=0)
    return out
